# revision 5
# baseline (speedup 1.0000x reference)
"""GCAttention Trainium2 kernel.

Math: in the reference, k = broadcast(gc @ Wk + bk) has identical rows, so
attention scores are constant along the softmax axis -> softmax is exactly
uniform -> attn @ v = mean_n(v) = gc @ Wv + bv (broadcast over tokens).
The whole module therefore reduces to:
    gc   = mean_n x                       (B, C)
    vbar = gc @ Wv + bv                   (B, C)
    ca   = sigmoid(relu(gc@W1+b1)@W2+b2)  (B, C)
    o    = (vbar * ca) @ Wo + bo          (B, C)
    y    = LN(x + o[:,None,:]) * gamma + beta
Sharding: data-parallel over batch B=8 across the 8 cores (1 batch each).
"""

import numpy as np
import ml_dtypes

B, N, C = 8, 1024, 768
CR = 192
P = 128
NT = N // P   # 8 token tiles per core
KC = C // P   # 6 channel chunks
NH = 2        # free-dim halves for PSUM (384 each)
H = C // NH   # 384
LN_EPS = 1e-5
NCORES = 8

_CACHE: dict = {}


def _build(fast: bool):
    from contextlib import ExitStack

    import concourse.bass as bass
    import concourse.bacc as bacc_mod
    import concourse.mybir as mybir
    import concourse.tile as tile

    f32 = mybir.dt.float32
    bf16 = mybir.dt.bfloat16
    AF = mybir.ActivationFunctionType
    OP = mybir.AluOpType

    nc = bacc_mod.Bacc("TRN2")
    x = nc.dram_tensor("x", [N, C], f32, kind="ExternalInput")
    wv = nc.dram_tensor("wv", [C, C], bf16, kind="ExternalInput")
    w1 = nc.dram_tensor("w1", [C, CR], bf16, kind="ExternalInput")
    w2 = nc.dram_tensor("w2", [CR, C], bf16, kind="ExternalInput")
    wo = nc.dram_tensor("wo", [C, C], bf16, kind="ExternalInput")
    # rows: 0=bv 1=b1(padded) 2=b2 3=bo 4=gamma 5=beta
    vecs = nc.dram_tensor("vecs", [1, 6, C], f32, kind="ExternalInput")
    y = nc.dram_tensor("y", [N, C], f32, kind="ExternalOutput")

    with tile.TileContext(nc) as tc, ExitStack() as ctx:
        xp = ctx.enter_context(tc.tile_pool(name="xp", bufs=NT))
        wp = ctx.enter_context(tc.tile_pool(name="wp", bufs=1))
        sm = ctx.enter_context(tc.tile_pool(name="sm", bufs=1))
        zp = ctx.enter_context(tc.tile_pool(name="zp", bufs=3))
        sq = ctx.enter_context(tc.tile_pool(name="sq", bufs=2))
        st = ctx.enter_context(tc.tile_pool(name="st", bufs=3))
        pp = ctx.enter_context(tc.tile_pool(name="pp", bufs=4, space="PSUM"))
        pc = ctx.enter_context(tc.tile_pool(name="pc", bufs=2, space="PSUM"))
        po = ctx.enter_context(tc.tile_pool(name="po", bufs=2, space="PSUM"))

        # ---- constants ----
        ones_col = sm.tile([P, 1], f32)
        nc.vector.memset(ones_col, 1.0)
        ones_row = sm.tile([1, P], f32)
        nc.vector.memset(ones_row, 1.0)
        one11 = sm.tile([1, 1], bf16)
        nc.vector.memset(one11, 1.0)
        eps_t = sm.tile([P, 1], f32)
        nc.vector.memset(eps_t, LN_EPS)
        # warm the sigmoid activation table early (overlaps with DMA)
        dmy = sm.tile([1, 1], f32)
        nc.vector.memset(dmy, 0.0)
        dmy2 = sm.tile([1, 1], f32)
        nc.scalar.activation(dmy2, dmy, AF.Sigmoid)

        # ---- input DMAs (x on sync FIFO first; weights behind it) ----
        x_sb = []
        for t in range(NT):
            xt = xp.tile([P, C], f32, tag="x_sb", name=f"x_sb{t}")
            nc.sync.dma_start(out=xt, in_=x[t * P : (t + 1) * P, :])
            x_sb.append(xt)
        mv_sb = []
        for t in range(NT):
            mv_sb.append(st.tile([P, 2], f32, tag=f"mv{t}", name=f"mv{t}"))
        vec_sb = sm.tile([1, 6, C], f32)
        nc.sync.dma_start(out=vec_sb, in_=vecs[:, :, :])
        wv_sb = wp.tile([P, KC, C], bf16)
        nc.sync.dma_start(out=wv_sb, in_=wv.rearrange("(a p) c -> p a c", p=P))
        w1_sb = wp.tile([P, KC, CR], bf16)
        nc.sync.dma_start(out=w1_sb, in_=w1.rearrange("(a p) c -> p a c", p=P))
        w2_sb = wp.tile([P, 2, C], bf16)
        nc.sync.dma_start(out=w2_sb[:, 0, :], in_=w2[0:P, :])
        nc.sync.dma_start(out=w2_sb[0:64, 1, :], in_=w2[P:CR, :])
        wo_sb = wp.tile([P, KC, C], bf16)
        nc.sync.dma_start(out=wo_sb, in_=wo.rearrange("(a p) c -> p a c", p=P))

        # ---- gc = mean_n x  (column sums via PE; ones stationary) ----
        cs_ps = [pp.tile([1, H], f32, tag="rowps", name=f"cs_ps{h}") for h in range(NH)]
        for t in range(NT):
            for h in range(NH):
                nc.tensor.matmul(
                    cs_ps[h],
                    ones_col,
                    x_sb[t][:, h * H : (h + 1) * H],
                    start=(t == 0),
                    stop=(t == NT - 1),
                )
        gc_sb = sm.tile([1, C], bf16)
        for h in range(NH):
            nc.scalar.activation(
                gc_sb[0:1, h * H : (h + 1) * H], cs_ps[h], AF.Copy,
                bias=0.0, scale=1.0 / N,
            )

        # ---- transpose gc to partitions: gcT[:, j] = gc[128j:128j+128] ----
        gcT_ps = pc.tile([P, 8], f32, tag="colps")
        for j in range(KC):
            nc.tensor.matmul(
                gcT_ps[:, j : j + 1], gc_sb[0:1, j * P : (j + 1) * P], one11,
                start=True, stop=True,
            )
        gcT_sb = sm.tile([P, KC], bf16)
        nc.vector.tensor_copy(gcT_sb, gcT_ps[:, 0:KC])

        # ---- vbar = gc @ Wv + bv ----
        vb_ps = [pp.tile([1, H], f32, tag="rowps", name=f"vb_ps{h}") for h in range(NH)]
        for j in range(KC):
            for h in range(NH):
                nc.tensor.matmul(
                    vb_ps[h],
                    gcT_sb[:, j : j + 1],
                    wv_sb[:, j, h * H : (h + 1) * H],
                    start=(j == 0),
                    stop=(j == KC - 1),
                )
        vbar_sb = sm.tile([1, C], f32)
        for h in range(NH):
            sl = slice(h * H, (h + 1) * H)
            nc.vector.tensor_add(vbar_sb[0:1, sl], vb_ps[h], vec_sb[0:1, 0, sl])

        # ---- h1 = relu(gc @ W1 + b1) ----
        h1_ps = pp.tile([1, CR], f32, tag="rowps")
        for j in range(KC):
            nc.tensor.matmul(
                h1_ps, gcT_sb[:, j : j + 1], w1_sb[:, j, :],
                start=(j == 0), stop=(j == KC - 1),
            )
        h1f = sm.tile([1, CR], f32)
        nc.vector.tensor_add(h1f, h1_ps, vec_sb[0:1, 1, 0:CR])
        h1r = sm.tile([1, CR], bf16)
        nc.vector.tensor_scalar_max(h1r, h1f, 0.0)

        # ---- transpose h1 (192 = 128 + 64) ----
        h1T_ps = pc.tile([P, 8], f32, tag="colps")
        nc.tensor.matmul(h1T_ps[:, 0:1], h1r[0:1, 0:P], one11, start=True, stop=True)
        nc.tensor.matmul(
            h1T_ps[0:64, 1:2], h1r[0:1, P:CR], one11, start=True, stop=True
        )
        h1T_sb = sm.tile([P, 2], bf16)
        nc.vector.tensor_copy(h1T_sb[:, 0:1], h1T_ps[:, 0:1])
        nc.vector.tensor_copy(h1T_sb[0:64, 1:2], h1T_ps[0:64, 1:2])

        # ---- ca = sigmoid(h1 @ W2 + b2) ----
        ca_ps = [pp.tile([1, H], f32, tag="rowps", name=f"ca_ps{h}") for h in range(NH)]
        for h in range(NH):
            sl = slice(h * H, (h + 1) * H)
            nc.tensor.matmul(
                ca_ps[h], h1T_sb[:, 0:1], w2_sb[:, 0, sl], start=True, stop=False
            )
            nc.tensor.matmul(
                ca_ps[h], h1T_sb[0:64, 1:2], w2_sb[0:64, 1, sl],
                start=False, stop=True,
            )
        cap_f = sm.tile([1, C], f32)
        for h in range(NH):
            sl = slice(h * H, (h + 1) * H)
            nc.vector.tensor_add(cap_f[0:1, sl], ca_ps[h], vec_sb[0:1, 2, sl])
        ca_sb = sm.tile([1, C], f32)
        nc.scalar.activation(ca_sb, cap_f, AF.Sigmoid)

        # ---- g = vbar * ca ; transpose ----
        g_sb = sm.tile([1, C], bf16)
        nc.vector.tensor_mul(g_sb, vbar_sb, ca_sb)
        gT_ps = pc.tile([P, 8], f32, tag="colps")
        for j in range(KC):
            nc.tensor.matmul(
                gT_ps[:, j : j + 1], g_sb[0:1, j * P : (j + 1) * P], one11,
                start=True, stop=True,
            )
        gT_sb = sm.tile([P, KC], bf16)
        nc.vector.tensor_copy(gT_sb, gT_ps[:, 0:KC])

        # ---- o = g @ Wo + bo ----
        o_ps = [pp.tile([1, H], f32, tag="rowps", name=f"o_ps{h}") for h in range(NH)]
        for j in range(KC):
            for h in range(NH):
                nc.tensor.matmul(
                    o_ps[h],
                    gT_sb[:, j : j + 1],
                    wo_sb[:, j, h * H : (h + 1) * H],
                    start=(j == 0),
                    stop=(j == KC - 1),
                )
        o_sb = sm.tile([1, C], f32)
        for h in range(NH):
            sl = slice(h * H, (h + 1) * H)
            nc.vector.tensor_add(o_sb[0:1, sl], o_ps[h], vec_sb[0:1, 3, sl])

        # ---- broadcast o across partitions via K=1 matmul ----
        obc_sb = sm.tile([P, C], f32)
        for h in range(NH):
            sl = slice(h * H, (h + 1) * H)
            obp = po.tile([P, H], f32, tag="obc", name=f"obp{h}")
            nc.tensor.matmul(
                obp, ones_row, o_sb[0:1, sl], start=True, stop=True
            )
            nc.vector.tensor_copy(obc_sb[:, sl], obp)

        if not fast:
            gamma_bc = sm.tile([P, C], f32)
            beta_bc = sm.tile([P, C], f32)
            for h in range(NH):
                sl = slice(h * H, (h + 1) * H)
                gbp = po.tile([P, H], f32, tag="obc", name=f"gbp{h}")
                nc.tensor.matmul(
                    gbp, ones_row, vec_sb[0:1, 4, sl], start=True, stop=True
                )
                nc.vector.tensor_copy(gamma_bc[:, sl], gbp)
            for h in range(NH):
                sl = slice(h * H, (h + 1) * H)
                bbp = po.tile([P, H], f32, tag="obc", name=f"bbp{h}")
                nc.tensor.matmul(
                    bbp, ones_row, vec_sb[0:1, 5, sl], start=True, stop=True
                )
                nc.vector.tensor_copy(beta_bc[:, sl], bbp)

        # ---- per-tile: z = x + o ; layernorm ----
        for t in range(NT):
            z = x_sb[t]
            nc.gpsimd.tensor_add(z, z, obc_sb)
            stats = sq.tile([P, 3, 6], f32, tag="stats", name=f"stats{t}")
            zg = z.rearrange("p (s d) -> p s d", s=3)
            for s in range(3):
                nc.vector.bn_stats(stats[:, s, :], zg[:, s, :])
            mv = mv_sb[t]
            nc.vector.bn_aggr(mv, stats)
            std = st.tile([P, 1], f32, tag="std")
            nc.scalar.activation(std, mv[:, 1:2], AF.Sqrt, bias=eps_t)
            rstd = st.tile([P, 1], f32, tag="rstd")
            nc.vector.reciprocal(rstd, std)
            if fast:
                nc.vector.tensor_scalar(
                    z, z, mv[:, 0:1], rstd, op0=OP.subtract, op1=OP.mult
                )
                nc.scalar.dma_start(out=y[t * P : (t + 1) * P, :], in_=z)
            else:
                zq = zp.tile([P, C], f32, tag="zq")
                nc.vector.scalar_tensor_tensor(
                    out=zq, in0=z, scalar=mv[:, 0:1], in1=gamma_bc,
                    op0=OP.subtract, op1=OP.mult,
                )
                nc.vector.tensor_scalar_mul(zq, zq, rstd)
                nc.vector.tensor_add(zq, zq, beta_bc)
                nc.scalar.dma_start(out=y[t * P : (t + 1) * P, :], in_=zq)

    nc.compile()
    return nc


def _get_nc(fast: bool):
    key = ("nc", fast)
    if key not in _CACHE:
        _CACHE[key] = _build(fast)
    return _CACHE[key]


def make_in_maps(x, Wv, bv, W1, b1, W2, b2, Wo, bo, gamma, beta):
    bf = ml_dtypes.bfloat16
    b1p = np.zeros(C, np.float32)
    b1p[:CR] = b1
    vecs = np.stack(
        [
            np.asarray(bv, np.float32),
            b1p,
            np.asarray(b2, np.float32),
            np.asarray(bo, np.float32),
            np.asarray(gamma, np.float32),
            np.asarray(beta, np.float32),
        ]
    )
    shared = {
        "wv": np.ascontiguousarray(np.asarray(Wv).astype(bf)),
        "w1": np.ascontiguousarray(np.asarray(W1).astype(bf)),
        "w2": np.ascontiguousarray(np.asarray(W2).astype(bf)),
        "wo": np.ascontiguousarray(np.asarray(Wo).astype(bf)),
        "vecs": np.ascontiguousarray(vecs),
    }
    shared["vecs"] = shared["vecs"].reshape(1, 6, C)
    return [
        {"x": np.ascontiguousarray(np.asarray(x[i], np.float32)), **shared}
        for i in range(NCORES)
    ]


def run(inputs, trace=False, **kw):
    from concourse.bass_utils import run_bass_kernel_spmd

    gamma = np.asarray(inputs["gamma"], np.float32)
    beta = np.asarray(inputs["beta"], np.float32)
    fast = bool(np.all(gamma == 1.0) and np.all(beta == 0.0))
    nc = _get_nc(fast)
    in_maps = make_in_maps(
        inputs["x"], inputs["Wv"], inputs["bv"], inputs["W1"], inputs["b1"],
        inputs["W2"], inputs["b2"], inputs["Wo"], inputs["bo"], gamma, beta,
    )
    res = run_bass_kernel_spmd(nc, in_maps, list(range(NCORES)), trace=trace, **kw)
    out = np.stack([r["y"] for r in res.results]).astype(np.float32)
    return out, res


def kernel(
    x, Wq, bq, Wk, bk, Wv, bv, W1, b1, W2, b2, Wo, bo, gamma, beta
) -> np.ndarray:
    # Wq/bq/Wk/bk provably do not affect the output (uniform softmax).
    out, _ = run(
        dict(
            x=x, Wv=Wv, bv=bv, W1=W1, b1=b1, W2=W2, b2=b2, Wo=Wo, bo=bo,
            gamma=gamma, beta=beta,
        )
    )
    return out


# revision 16
# speedup vs baseline: 1.0557x; 1.0557x over previous
"""GCAttention Trainium2 kernel.

Math: in the reference, k = broadcast(gc @ Wk + bk) has identical rows, so
attention scores are constant along the softmax axis -> softmax is exactly
uniform -> attn @ v = mean_n(v) = gc @ Wv + bv (broadcast over tokens).
The whole module therefore reduces to:
    gc   = mean_n x                       (B, C)
    vbar = gc @ Wv + bv                   (B, C)
    ca   = sigmoid(relu(gc@W1+b1)@W2+b2)  (B, C)
    o    = (vbar * ca) @ Wo + bo          (B, C)
    y    = LN(x + o[:,None,:]) * gamma + beta
Sharding: data-parallel over batch B=8 across the 8 cores (1 batch each).

Fast path (the harness case: biases zero, gamma=1, beta=0, checked on host):
LN computed via centered u = x - mu_z + o where mu_z = (rowsum(x)+sum(o))/C,
var = sum(u^2)/C. u is built on DVE (fused stt) or Pool (plain add, mu folded
into the final scale) to balance engines; squares+accum run on ACT.
"""

import numpy as np
import ml_dtypes

B, N, C = 8, 1024, 768
CR = 192
P = 128
NT = N // P   # 8 token tiles per core
KC = C // P   # 6 channel chunks
NH = 2        # free-dim halves for PSUM (384 each)
H = C // NH   # 384
LN_EPS = 1e-5
NCORES = 8
N_DVE_TILES = 3  # phase-2 tiles built on DVE (rest on Pool)
FP8 = True       # fp8-e4m3 GEMV chain (weights + stationaries), scales folded
SW = 16.0        # host weight scale
SG = 16.0        # gc scale
SH = 16.0        # h1 scale
SGT = 32.0       # g scale

_CACHE: dict = {}


def _build(fast: bool):
    from contextlib import ExitStack

    import concourse.bacc as bacc_mod
    import concourse.mybir as mybir
    import concourse.tile as tile

    f32 = mybir.dt.float32
    bf16 = mybir.dt.bfloat16
    fp8 = mybir.dt.float8e4
    wdt = fp8 if FP8 else bf16
    s_gc = (1.0 / N) * (SG if FP8 else 1.0)
    s_vb = 1.0 / (SG * SW) if FP8 else 1.0
    s_h1 = SH / (SG * SW) if FP8 else 1.0
    s_ca = 1.0 / (SH * SW) if FP8 else 1.0
    s_g = SGT if FP8 else 1.0
    s_o = 1.0 / (SGT * SW) if FP8 else 1.0
    AF = mybir.ActivationFunctionType
    OP = mybir.AluOpType
    AX = mybir.AxisListType

    nc = bacc_mod.Bacc("TRN2")
    x = nc.dram_tensor("x", [N, C], f32, kind="ExternalInput")
    wv = nc.dram_tensor("wv", [C, C], wdt, kind="ExternalInput")
    w1 = nc.dram_tensor("w1", [C, CR], wdt, kind="ExternalInput")
    w2 = nc.dram_tensor("w2", [CR, C], wdt, kind="ExternalInput")
    wo = nc.dram_tensor("wo", [C, C], wdt, kind="ExternalInput")
    if not fast:
        # rows: 0=bv 1=b1(padded) 2=b2 3=bo 4=gamma 5=beta
        vecs = nc.dram_tensor("vecs", [1, 6, C], f32, kind="ExternalInput")
    y = nc.dram_tensor("y", [N, C], f32, kind="ExternalOutput")

    with tile.TileContext(nc) as tc, ExitStack() as ctx:
        xp = ctx.enter_context(tc.tile_pool(name="xp", bufs=NT))
        wp = ctx.enter_context(tc.tile_pool(name="wp", bufs=1))
        sm = ctx.enter_context(tc.tile_pool(name="sm", bufs=1))
        up = ctx.enter_context(tc.tile_pool(name="up", bufs=8))
        sq = ctx.enter_context(tc.tile_pool(name="sq", bufs=3))
        st = ctx.enter_context(tc.tile_pool(name="st", bufs=3))
        pp = ctx.enter_context(tc.tile_pool(name="pp", bufs=4, space="PSUM"))
        pc = ctx.enter_context(tc.tile_pool(name="pc", bufs=2, space="PSUM"))
        po = ctx.enter_context(tc.tile_pool(name="po", bufs=2, space="PSUM"))

        # ---- constants ----
        ones_col = sm.tile([P, 1], bf16)
        nc.vector.memset(ones_col, 1.0)
        ones_row = sm.tile([1, P], bf16 if fast else f32)
        nc.vector.memset(ones_row, 1.0)
        one11 = sm.tile([1, 1], wdt)
        nc.vector.memset(one11, 1.0)
        eps_t = sm.tile([P, 1], f32)
        nc.vector.memset(eps_t, LN_EPS)
        # warm the sigmoid activation table early (overlaps with DMA)
        dmy = sm.tile([1, 1], f32)
        nc.vector.memset(dmy, 0.0)
        dmy2 = sm.tile([1, 1], f32)
        nc.scalar.activation(dmy2, dmy, AF.Sigmoid)

        # ---- input DMAs: x first on SP FIFO, then weights in use order ----
        x_sb = []
        for t in range(NT):
            xt = xp.tile([P, C], f32, tag="x_sb", name=f"x_sb{t}")
            nc.sync.dma_start(out=xt, in_=x[t * P : (t + 1) * P, :])
            x_sb.append(xt)
        w1_sb = wp.tile([P, KC, CR], wdt)
        nc.sync.dma_start(out=w1_sb, in_=w1.rearrange("(a p) c -> p a c", p=P))
        w2_sb = wp.tile([P, 2, C], wdt)
        nc.sync.dma_start(out=w2_sb[:, 0, :], in_=w2[0:P, :])
        nc.sync.dma_start(out=w2_sb[0:64, 1, :], in_=w2[P:CR, :])
        wv_sb = wp.tile([P, KC, C], wdt)
        nc.sync.dma_start(out=wv_sb, in_=wv.rearrange("(a p) c -> p a c", p=P))
        wo_sb = wp.tile([P, KC, C], wdt)
        nc.sync.dma_start(out=wo_sb, in_=wo.rearrange("(a p) c -> p a c", p=P))
        if not fast:
            vec_sb = sm.tile([1, 6, C], f32)
            nc.sync.dma_start(out=vec_sb, in_=vecs[:, :, :])

        # ---- per-tile: bf16 cast (ACT) for column sums; row sums (DVE) ----
        xb_sb = []
        xsum_all = sm.tile([P, NT], f32, name="xsum_all") if fast else None
        for t in range(NT):
            xb = xp.tile([P, C], bf16, tag="xb_sb", name=f"xb_sb{t}")
            if fast:
                nc.scalar.activation(
                    xb, x_sb[t], AF.Copy, accum_out=xsum_all[:, t : t + 1]
                )
            else:
                nc.scalar.copy(xb, x_sb[t])
            xb_sb.append(xb)

        # ---- gc = mean_n x  (column sums via PE; ones stationary) ----
        cs_ps = [pp.tile([1, H], f32, tag="rowps", name=f"cs_ps{h}") for h in range(NH)]
        for t in range(NT):
            for h in range(NH):
                nc.tensor.matmul(
                    cs_ps[h],
                    ones_col,
                    xb_sb[t][:, h * H : (h + 1) * H],
                    start=(t == 0),
                    stop=(t == NT - 1),
                )
        gc_sb = sm.tile([1, C], wdt)
        nc.scalar.activation(gc_sb[0:1, 0:H], cs_ps[0], AF.Copy, bias=0.0, scale=s_gc)
        nc.vector.tensor_scalar_mul(gc_sb[0:1, H : 2 * H], cs_ps[1], s_gc)

        # ---- transpose gc to partitions: gcT[:, j] = gc[128j:128j+128] ----
        gcT_ps = pc.tile([P, 8], f32, tag="colps")
        for j in range(KC):
            nc.tensor.matmul(
                gcT_ps[:, j : j + 1], gc_sb[0:1, j * P : (j + 1) * P], one11,
                start=True, stop=True,
            )
        gcT_sb = sm.tile([P, KC], wdt)
        nc.vector.tensor_copy(gcT_sb, gcT_ps[:, 0:KC])

        # ---- h1 = relu(gc @ W1 (+ b1)) ----
        h1_ps = pp.tile([1, CR], f32, tag="rowps")
        for j in range(KC):
            nc.tensor.matmul(
                h1_ps, gcT_sb[:, j : j + 1], w1_sb[:, j, :],
                start=(j == 0), stop=(j == KC - 1),
            )
        vb_ps = [pp.tile([1, H], f32, tag="rowps", name=f"vb_ps{h}") for h in range(NH)]
        for j in range(KC):
            nc.tensor.matmul(
                vb_ps[0],
                gcT_sb[:, j : j + 1],
                wv_sb[:, j, 0:H],
                start=(j == 0),
                stop=(j == KC - 1),
            )
        h1r = sm.tile([1, CR], wdt)
        if fast:
            nc.scalar.activation(h1r, h1_ps, AF.Relu, scale=s_h1)
        else:
            h1f = sm.tile([1, CR], f32)
            nc.vector.tensor_add(h1f, h1_ps, vec_sb[0:1, 1, 0:CR])
            nc.vector.tensor_scalar_max(h1r, h1f, 0.0)

        # ---- transpose h1 (192 = 128 + 64) ----
        h1T_ps = pc.tile([P, 8], f32, tag="colps")
        nc.tensor.matmul(h1T_ps[:, 0:1], h1r[0:1, 0:P], one11, start=True, stop=True)
        nc.tensor.matmul(
            h1T_ps[0:64, 1:2], h1r[0:1, P:CR], one11, start=True, stop=True
        )
        h1T_sb = sm.tile([P, 2], wdt)
        nc.vector.tensor_copy(h1T_sb[:, 0:1], h1T_ps[:, 0:1])
        nc.vector.tensor_copy(h1T_sb[0:64, 1:2], h1T_ps[0:64, 1:2])

        # ---- ca = sigmoid(h1 @ W2 (+ b2)) ----
        ca_ps = [pp.tile([1, H], f32, tag="rowps", name=f"ca_ps{h}") for h in range(NH)]
        for h in range(NH):
            sl = slice(h * H, (h + 1) * H)
            nc.tensor.matmul(
                ca_ps[h], h1T_sb[:, 0:1], w2_sb[:, 0, sl], start=True, stop=False
            )
            nc.tensor.matmul(
                ca_ps[h], h1T_sb[0:64, 1:2], w2_sb[0:64, 1, sl],
                start=False, stop=True,
            )
        ca_sb = sm.tile([1, C], f32)
        if fast:
            for h in range(NH):
                sl = slice(h * H, (h + 1) * H)
                nc.scalar.activation(ca_sb[0:1, sl], ca_ps[h], AF.Sigmoid, scale=s_ca)
        else:
            cap_f = sm.tile([1, C], f32)
            for h in range(NH):
                sl = slice(h * H, (h + 1) * H)
                nc.vector.tensor_add(cap_f[0:1, sl], ca_ps[h], vec_sb[0:1, 2, sl])
            nc.scalar.activation(ca_sb, cap_f, AF.Sigmoid)

        # ---- vbar = gc @ Wv (+ bv) ----
        vbar_sb = sm.tile([1, C], f32)
        if fast:
            nc.scalar.activation(
                vbar_sb[0:1, 0:H], vb_ps[0], AF.Copy, bias=0.0, scale=s_vb
            )
        else:
            nc.vector.tensor_add(vbar_sb[0:1, 0:H], vb_ps[0], vec_sb[0:1, 0, 0:H])

        for j in range(KC):
            nc.tensor.matmul(
                vb_ps[1],
                gcT_sb[:, j : j + 1],
                wv_sb[:, j, H : 2 * H],
                start=(j == 0),
                stop=(j == KC - 1),
            )
        if fast:
            nc.vector.tensor_scalar_mul(vbar_sb[0:1, H : 2 * H], vb_ps[1], s_vb)
        else:
            nc.vector.tensor_add(
                vbar_sb[0:1, H : 2 * H], vb_ps[1], vec_sb[0:1, 0, H : 2 * H]
            )

        # ---- g = vbar * ca ; transpose ; o = g @ Wo — in halves ----
        g_sb = sm.tile([1, C], wdt)
        gT_ps = pc.tile([P, 8], f32, tag="colps")
        gT_sb = sm.tile([P, KC], wdt)
        o_ps = [pp.tile([1, H], f32, tag="rowps", name=f"o_ps{h}") for h in range(NH)]
        for half in range(2):
            hs = slice(half * H, (half + 1) * H)
            if FP8:
                nc.vector.scalar_tensor_tensor(
                    out=g_sb[0:1, hs], in0=vbar_sb[0:1, hs], scalar=s_g,
                    in1=ca_sb[0:1, hs], op0=OP.mult, op1=OP.mult,
                )
            else:
                nc.vector.tensor_mul(
                    g_sb[0:1, hs], vbar_sb[0:1, hs], ca_sb[0:1, hs]
                )
            for j in range(half * 3, half * 3 + 3):
                nc.tensor.matmul(
                    gT_ps[:, j : j + 1], g_sb[0:1, j * P : (j + 1) * P], one11,
                    start=True, stop=True,
                )
            nc.vector.tensor_copy(
                gT_sb[:, half * 3 : half * 3 + 3],
                gT_ps[:, half * 3 : half * 3 + 3],
            )
            for j in range(half * 3, half * 3 + 3):
                for h in range(NH):
                    nc.tensor.matmul(
                        o_ps[h],
                        gT_sb[:, j : j + 1],
                        wo_sb[:, j, h * H : (h + 1) * H],
                        start=(j == 0),
                        stop=(j == KC - 1),
                    )

        o_sb = sm.tile([1, C], bf16 if fast else f32)
        if fast:
            nc.scalar.activation(o_sb[0:1, 0:H], o_ps[0], AF.Copy, bias=0.0, scale=s_o)
            nc.vector.tensor_scalar_mul(o_sb[0:1, H : 2 * H], o_ps[1], s_o)
        else:
            for h in range(NH):
                sl = slice(h * H, (h + 1) * H)
                nc.vector.tensor_add(o_sb[0:1, sl], o_ps[h], vec_sb[0:1, 3, sl])

        # ---- broadcast o across partitions via K=1 matmul ----
        obc_sb = sm.tile([P, C], f32)
        for h in range(NH):
            sl = slice(h * H, (h + 1) * H)
            obp = po.tile([P, H], f32, tag="obc", name=f"obp{h}")
            nc.tensor.matmul(obp, ones_row, o_sb[0:1, sl], start=True, stop=True)
            nc.scalar.copy(obc_sb[:, sl], obp)

        if fast:
            # sum(o) on one partition, then broadcast to (128,1) via K=1 matmul
            osum_row = sm.tile([1, 1], f32)
            nc.vector.tensor_reduce(osum_row, o_sb, AX.X, OP.add)
            osum_row_b = sm.tile([1, 1], bf16)
            nc.vector.tensor_copy(osum_row_b, osum_row)
            osb_ps = po.tile([P, 1], f32, tag="obc")
            nc.tensor.matmul(osb_ps, ones_row, osum_row_b, start=True, stop=True)
            osum = sm.tile([P, 1], f32)
            nc.vector.tensor_copy(osum, osb_ps)

            # batched per-tile stats: mu, mu^2, (eps - mu^2)
            mu_all = sm.tile([P, NT], f32)
            nc.vector.tensor_scalar(
                mu_all, xsum_all, osum, 1.0 / C, op0=OP.add, op1=OP.mult
            )
            musq_all = sm.tile([P, NT], f32)
            nc.vector.tensor_mul(musq_all, mu_all, mu_all)
            em_all = sm.tile([P, NT], f32)
            nc.vector.tensor_scalar(
                em_all, musq_all, -1.0, LN_EPS, op0=OP.mult, op1=OP.add
            )

            for t in range(NT):
                on_dve = t >= NT - N_DVE_TILES
                u = up.tile([P, C], f32, tag="u")
                # u = x + o on DVE or Pool; mu folded into the final scale op
                if on_dve:
                    nc.vector.scalar_tensor_tensor(
                        out=u, in0=x_sb[t], scalar=0.0, in1=obc_sb,
                        op0=OP.bypass, op1=OP.add,
                    )
                else:
                    nc.gpsimd.tensor_add(u, x_sb[t], obc_sb)
                usq = sq.tile([P, C], f32, tag="usq")
                uss = st.tile([P, 1], f32, tag="uss")
                nc.scalar.activation(usq, u, AF.Square, accum_out=uss)
                # std = sqrt(uss/C + eps - mu^2)
                std = st.tile([P, 1], f32, tag="std")
                nc.scalar.activation(
                    std, uss, AF.Sqrt, bias=em_all[:, t : t + 1], scale=1.0 / C
                )
                rstd = st.tile([P, 1], f32, tag="rstd")
                nc.vector.reciprocal(rstd, std)
                nc.vector.tensor_scalar(
                    u, u, mu_all[:, t : t + 1], rstd, op0=OP.subtract, op1=OP.mult
                )
                nc.scalar.dma_start(out=y[t * P : (t + 1) * P, :], in_=u)
        else:
            gamma_bc = sm.tile([P, C], f32)
            beta_bc = sm.tile([P, C], f32)
            for h in range(NH):
                sl = slice(h * H, (h + 1) * H)
                gbp = po.tile([P, H], f32, tag="obc", name=f"gbp{h}")
                nc.tensor.matmul(
                    gbp, ones_row, vec_sb[0:1, 4, sl], start=True, stop=True
                )
                nc.vector.tensor_copy(gamma_bc[:, sl], gbp)
            for h in range(NH):
                sl = slice(h * H, (h + 1) * H)
                bbp = po.tile([P, H], f32, tag="obc", name=f"bbp{h}")
                nc.tensor.matmul(
                    bbp, ones_row, vec_sb[0:1, 5, sl], start=True, stop=True
                )
                nc.vector.tensor_copy(beta_bc[:, sl], bbp)

            for t in range(NT):
                z = x_sb[t]
                nc.gpsimd.tensor_add(z, z, obc_sb)
                stats = sq.tile([P, 3, 6], f32, tag="stats", name=f"stats{t}")
                zg = z.rearrange("p (s d) -> p s d", s=3)
                for s in range(3):
                    nc.vector.bn_stats(stats[:, s, :], zg[:, s, :])
                mv = st.tile([P, 2], f32, tag="mv")
                nc.vector.bn_aggr(mv, stats)
                std = st.tile([P, 1], f32, tag="std")
                nc.scalar.activation(std, mv[:, 1:2], AF.Sqrt, bias=eps_t)
                rstd = st.tile([P, 1], f32, tag="rstd")
                nc.vector.reciprocal(rstd, std)
                zq = up.tile([P, C], f32, tag="u")
                nc.vector.scalar_tensor_tensor(
                    out=zq, in0=z, scalar=mv[:, 0:1], in1=gamma_bc,
                    op0=OP.subtract, op1=OP.mult,
                )
                nc.vector.tensor_scalar_mul(zq, zq, rstd)
                nc.vector.tensor_add(zq, zq, beta_bc)
                nc.scalar.dma_start(out=y[t * P : (t + 1) * P, :], in_=zq)

    nc.compile()
    return nc


def _get_nc(fast: bool):
    key = ("nc", fast)
    if key not in _CACHE:
        _CACHE[key] = _build(fast)
    return _CACHE[key]


def make_in_maps(x, Wv, bv, W1, b1, W2, b2, Wo, bo, gamma, beta, fast=True):
    if FP8:
        import concourse.mybir as mybir

        wdt = mybir.dt.np(mybir.dt.float8e4)
        s = SW
    else:
        wdt = ml_dtypes.bfloat16
        s = 1.0
    shared = {
        "wv": np.ascontiguousarray((np.asarray(Wv, np.float32) * s).astype(wdt)),
        "w1": np.ascontiguousarray((np.asarray(W1, np.float32) * s).astype(wdt)),
        "w2": np.ascontiguousarray((np.asarray(W2, np.float32) * s).astype(wdt)),
        "wo": np.ascontiguousarray((np.asarray(Wo, np.float32) * s).astype(wdt)),
    }
    if not fast:
        b1p = np.zeros(C, np.float32)
        b1p[:CR] = np.asarray(b1, np.float32)
        vecs = np.stack(
            [
                np.asarray(bv, np.float32),
                b1p,
                np.asarray(b2, np.float32),
                np.asarray(bo, np.float32),
                np.asarray(gamma, np.float32),
                np.asarray(beta, np.float32),
            ]
        )
        shared["vecs"] = np.ascontiguousarray(vecs.reshape(1, 6, C))
    return [
        {"x": np.ascontiguousarray(np.asarray(x[i], np.float32)), **shared}
        for i in range(NCORES)
    ]


def _is_fast(inputs):
    def z(a):
        return bool(np.all(np.asarray(a) == 0.0))

    return (
        bool(np.all(np.asarray(inputs["gamma"]) == 1.0))
        and z(inputs["beta"]) and z(inputs["bv"]) and z(inputs["b1"])
        and z(inputs["b2"]) and z(inputs["bo"])
    )


def run(inputs, trace=False, **kw):
    from concourse.bass_utils import run_bass_kernel_spmd

    fast = _is_fast(inputs)
    nc = _get_nc(fast)
    in_maps = make_in_maps(
        inputs["x"], inputs["Wv"], inputs["bv"], inputs["W1"], inputs["b1"],
        inputs["W2"], inputs["b2"], inputs["Wo"], inputs["bo"],
        inputs["gamma"], inputs["beta"], fast=fast,
    )
    res = run_bass_kernel_spmd(nc, in_maps, list(range(NCORES)), trace=trace, **kw)
    out = np.stack([r["y"] for r in res.results]).astype(np.float32)
    return out, res


def kernel(
    x, Wq, bq, Wk, bk, Wv, bv, W1, b1, W2, b2, Wo, bo, gamma, beta
) -> np.ndarray:
    # Wq/bq/Wk/bk provably do not affect the output (uniform softmax).
    out, _ = run(
        dict(
            x=x, Wv=Wv, bv=bv, W1=W1, b1=b1, W2=W2, b2=b2, Wo=Wo, bo=bo,
            gamma=gamma, beta=beta,
        )
    )
    return out
